# revision 23
# baseline (speedup 1.0000x reference)
"""ConvModLayer (StyleGAN2-style modulated 3x3 conv) on 8 Trainium2
NeuronCores — data-parallel over the batch (16 samples -> 2 per core).

Math (equivalent to the reference):
  cscale = 1/sqrt(512*9)
  s' = s * cscale
  sigma_sq[b,o] = sum_{i,ky,kx} (weight[o,i,ky,kx] * s'[b,i])^2
  out[b] = conv3x3(x[b] * s'[b,:,None,None], weight) * rsqrt(sigma_sq[b] + eps)

Device kernel (per core, identical SPMD program) — 1D Winograd F(2,3)
along x, direct conv along y, bf16 matmul operands:

  The kx-dimension (3 taps) is replaced by 4 Winograd products shared
  between each pair of output columns:
    V_v[c, y, bx] = BT-combos of padded x cols 2bx+0..3   (DVE, 4 ops/chunk)
    M_v[o, y, bx] = sum_{i,ky} U[o,i,ky,v] * V_v[i, y+ky, bx]  (PE, PSUM)
    out[:, 2bx+0] = (M0 + M1 + M2) * rsqrt(sigma)
    out[:, 2bx+1] = (M1 - M2 - M3) * rsqrt(sigma)
  MACs per output: 4*3*512 = 3072 vs direct 4608 -> 1.5x less PE time.

  U = w @ G^T (per ky) is sample-independent, computed on the HOST and
  shipped as a [128, 12(v*3+ky), 4, 512] bf16 input. The demod norm
  helper wsq[i,o] = sum_kykx w^2 is also host-computed (f32), so sigma
  on-device is just 16 tiny matmuls against s'^2 — no DVE reduction.

  PSUM: 4 M_v banks per 16-output-row group, two groups in flight.
"""

import sys
from contextlib import ExitStack

if "/opt/trn_rl_repo" not in sys.path:
    sys.path.insert(0, "/opt/trn_rl_repo")

import ml_dtypes
import numpy as np

import concourse.bacc as bacc
import concourse.mybir as mybir
import concourse.tile as tile
from concourse.bass_utils import run_bass_kernel_spmd

F32 = mybir.dt.float32
BF16 = mybir.dt.bfloat16

N_CORES = 8
B = 16
B2 = B // N_CORES  # samples per core
C = 512
NCH = 4  # 128-partition channel chunks
H = W = 64
EPS = 1e-8
CSCALE = 1.0 / (C * 9) ** 0.5

_NC_CACHE = {}


def _build(psum_bufs: int = 8, raw_bufs: int = 4):
    nc = bacc.Bacc("TRN2", target_bir_lowering=False, debug=False)

    x_d = nc.dram_tensor("x", [B2, C, H, W], BF16, kind="ExternalInput")
    s_d = nc.dram_tensor("s", [128, NCH, B2], F32, kind="ExternalInput")
    u_d = nc.dram_tensor("u", [128, NCH, 12, NCH, 128], BF16, kind="ExternalInput")
    wsq_d = nc.dram_tensor("wsq", [128, NCH, C], F32, kind="ExternalInput")
    o_d = nc.dram_tensor("o", [B2, C, H, W], F32, kind="ExternalOutput")

    with tile.TileContext(nc) as tc, ExitStack() as ctx:
        upool = ctx.enter_context(tc.tile_pool(name="upool", bufs=1))
        spool = ctx.enter_context(tc.tile_pool(name="spool", bufs=1))
        rawpool = ctx.enter_context(tc.tile_pool(name="rawpool", bufs=raw_bufs))
        ppool = ctx.enter_context(tc.tile_pool(name="ppool", bufs=1))
        vpool = ctx.enter_context(tc.tile_pool(name="vpool", bufs=2))
        tpool = ctx.enter_context(tc.tile_pool(name="tpool", bufs=2))
        opool = ctx.enter_context(tc.tile_pool(name="opool", bufs=4))
        pspool = ctx.enter_context(
            tc.tile_pool(name="pspool", bufs=psum_bufs, space="PSUM")
        )

        s_t = spool.tile([128, NCH, B2], F32)
        nc.sync.dma_start(s_t[:], s_d[:])
        nc.vector.tensor_scalar_mul(s_t[:], s_t[:], CSCALE)
        s2_t = spool.tile([128, NCH, B2], F32)
        nc.vector.tensor_mul(s2_t[:], s_t[:], s_t[:])

        # zeros for pad regions (f32 source for convert-copies)
        z66 = spool.tile([128, 66], F32)
        nc.vector.memset(z66[:], 0.0)

        def make_p(h, ic):
            """Padded modulated image tile [128, 34, 66] bf16: row zr and
            cols 0,65 are the conv zero-pad. Pad-zeroing runs on the
            (otherwise idle) GpSimd engine, off the DVE critical path."""
            zr = 0 if h == 0 else 33
            p = ppool.tile([128, 34, 66], BF16, tag=f"p{ic}", name="p")
            nc.gpsimd.memset(p[:, zr, :], 0.0)
            nc.gpsimd.memset(p[:, :, 0], 0.0)
            nc.gpsimd.memset(p[:, :, 65], 0.0)
            return p

        def scale_p(p, b, h, ic, raw, rows):
            # modulation scale on ACT, leaving DVE only the V transform
            r0, r1 = rows
            off = 1 if h == 0 else 0  # data rows start at this tile row
            nc.scalar.mul(
                p[:, off + r0 : off + r1, 1:65],
                raw[:, r0:r1, :],
                s_t[:, ic, b : b + 1],
            )

        def alloc_v(ic):
            return [
                vpool.tile([128, 34, 32], BF16, tag=f"v{ic}_{v}", name="vt")
                for v in range(4)
            ]

        def fill_v(vts, p, rows):
            """Winograd column transform: V_v[y, bx] over padded cols
            2bx+v'. v0=d0-d2, v1=d1+d2, v2=d2-d1, v3=d1-d3."""
            r0, r1 = rows
            d0 = p[:, r0:r1, 0:64:2]
            d1 = p[:, r0:r1, 1:65:2]
            d2 = p[:, r0:r1, 2:66:2]
            d3 = p[:, r0:r1, 3:66:2]
            nc.vector.tensor_sub(vts[0][:, r0:r1, :], d0, d2)
            nc.vector.tensor_add(vts[1][:, r0:r1, :], d1, d2)
            nc.vector.tensor_sub(vts[2][:, r0:r1, :], d2, d1)
            nc.vector.tensor_sub(vts[3][:, r0:r1, :], d1, d3)

        def prep_half(b, h, ics=tuple(range(NCH))):
            xts = {}
            for ic in ics:
                raw = rawpool.tile([128, 33, 64], BF16, tag="raw", name="raw")
                r0 = 0 if h == 0 else 31
                nc.sync.dma_start(
                    raw[:], x_d[b, ic * 128 : (ic + 1) * 128, r0 : r0 + 33, :]
                )
                p = make_p(h, ic)
                scale_p(p, b, h, ic, raw, (0, 33))
                vts = alloc_v(ic)
                fill_v(vts, p, (0, 34))
                xts[ic] = vts
            return xts

        # Two hw DMA queues: x chunks + out stores on the SP (sync)
        # queue, U/wsq weights on the Activation queue — so the first
        # conv groups aren't gated on 6.3 MB of U behind x traffic.
        # U is laid out oc-major on the host so each oc's weights are
        # ONE contiguous 1.5 MB DMA (big packets); oc0 lands first and
        # covers the first two groups while the rest streams behind.
        u_oc = [
            upool.tile([128, 12, NCH, 128], BF16, tag=f"u{oc}", name="ut")
            for oc in range(NCH)
        ]
        # oc0 in three sub-DMAs so the first matmul gates on only the
        # first 0.5 MB rather than the full 1.5 MB transfer
        for vk0 in range(0, 12, 4):
            nc.scalar.dma_start(
                u_oc[0][:, vk0 : vk0 + 4], u_d[:, 0, vk0 : vk0 + 4]
            )
        wsq_t = spool.tile([128, NCH, C], F32)
        nc.scalar.dma_start(wsq_t[:], wsq_d[:])
        for oc in range(1, NCH):
            nc.scalar.dma_start(u_oc[oc][:], u_d[:, oc])

        # x chunk 0 rows 0..16 arrive first (the first matmul's gate);
        # rows 17..32 are only needed by the second group (~22us in), so
        # their DMA queues BEHIND chunks 1 and 2
        raw0 = rawpool.tile([128, 33, 64], BF16, tag="raw", name="raw")
        nc.sync.dma_start(raw0[:, 0:17], x_d[0, 0:128, 0:17, :])
        p0 = make_p(0, 0)
        scale_p(p0, 0, 0, 0, raw0, (0, 17))
        vts0 = alloc_v(0)
        fill_v(vts0, p0, (0, 18))
        xts_00 = {0: vts0}
        xts_00.update(prep_half(0, 0, ics=(1, 2)))
        nc.sync.dma_start(raw0[:, 17:33], x_d[0, 0:128, 17:33, :])
        xts_00.update(prep_half(0, 0, ics=(3,)))
        scale_p(p0, 0, 0, 0, raw0, (17, 33))
        fill_v(vts0, p0, (18, 34))

        # ---- sigma_sq[b, o] = sum_i wsq[i,o] * s'2[i,b] ----
        sig_t = spool.tile([128, NCH, B2], F32)

        def emit_sigma():
            # borrow a conv PSUM bank from the pool ring (same tag/shape)
            psig_t = pspool.tile([128, 16, 32], F32, tag="m", name="psig")
            for ic in range(NCH):
                for oc in range(NCH):
                    # start=True clears the WHOLE bank -> only the global
                    # first matmul sets it; later groups overwrite-where-
                    # unset via per-element has_written bits.
                    nc.tensor.matmul(
                        psig_t[:, oc, 0:B2],
                        wsq_t[:, ic, oc * 128 : (oc + 1) * 128],
                        s2_t[:, ic, :],
                        start=(ic == 0 and oc == 0),
                        stop=(ic == 3 and oc == 3),
                        skip_group_check=True,
                    )
            nc.vector.tensor_scalar_add(sig_t[:], psig_t[:, 0:NCH, 0:B2], EPS)
            nc.scalar.sqrt(sig_t[:], sig_t[:])
            nc.vector.reciprocal(sig_t[:], sig_t[:])

        # ---- conv: per sample, 2 halves of 32 rows = 2 groups of 16 ----
        quarters = [(b, h) for b in range(B2) for h in range(2)]
        preps = {0: xts_00}

        def emit_out(b, h, oc, yb2, accs, last=False, gi=[0]):
            # accs = [M0..M3] in PSUM, each [128, 16 rows, 32 bx]
            sig = sig_t[:, oc, b : b + 1]
            y0 = h * 32 + yb2 * 16
            # the very last group drains in two row-halves so the first
            # half's store overlaps the second half's transform
            halves = ((0, 8), (8, 16)) if last else ((0, 16),)
            for r0, r1 in halves:
                rr = r1 - r0
                c1 = tpool.tile([128, 16, 32], F32, tag="c1", name="c1")
                e1 = tpool.tile([128, 16, 32], F32, tag="e1", name="e1")
                e2 = tpool.tile([128, 16, 32], F32, tag="e2", name="e2")
                o1 = tpool.tile([128, 16, 32], F32, tag="o1", name="o1")
                o2 = tpool.tile([128, 16, 32], F32, tag="o2", name="o2")
                out_t = opool.tile([128, 16, 64], F32, tag="out", name="out")
                nc.vector.tensor_copy(c1[:, 0:rr, :], accs[1][:, r0:r1, :])
                nc.vector.tensor_add(e1[:, 0:rr, :], accs[0][:, r0:r1, :], c1[:, 0:rr, :])
                nc.vector.tensor_add(e2[:, 0:rr, :], e1[:, 0:rr, :], accs[2][:, r0:r1, :])
                nc.vector.tensor_sub(o1[:, 0:rr, :], c1[:, 0:rr, :], accs[2][:, r0:r1, :])
                nc.vector.tensor_sub(o2[:, 0:rr, :], o1[:, 0:rr, :], accs[3][:, r0:r1, :])
                nc.scalar.mul(out_t[:, 0:rr, 0::2], e2[:, 0:rr, :], sig)
                nc.scalar.mul(out_t[:, 0:rr, 1::2], o2[:, 0:rr, :], sig)
                # alternate store queues so out DMAs never back up behind
                # the next quarter's x loads (SP) or the U stream (ACT)
                eng = nc.sync if gi[0] % 2 == 0 else nc.scalar
                gi[0] += 1
                eng.dma_start(
                    o_d[b, oc * 128 : (oc + 1) * 128, y0 + r0 : y0 + r1, :],
                    out_t[:, 0:rr, :],
                )

        for qi, (b, h) in enumerate(quarters):
            xts = preps.pop(qi)
            for oc in range(NCH):
                for yb2 in range(2):
                    accs = [
                        pspool.tile([128, 16, 32], F32, tag="m", name=f"m{v}")
                        for v in range(4)
                    ]
                    # first two groups run ic-outer so early matmuls
                    # lean on x chunks that have already arrived; the
                    # steady state runs v-outer
                    if qi == 0 and oc == 0:
                        order = [
                            (v, ky, ic)
                            for ic in range(NCH)
                            for v in range(4)
                            for ky in range(3)
                        ]
                    else:
                        order = [
                            (v, ky, ic)
                            for v in range(4)
                            for ky in range(3)
                            for ic in range(NCH)
                        ]
                    for v, ky, ic in order:
                        nc.tensor.matmul(
                            accs[v][:],
                            u_oc[oc][:, v * 3 + ky, ic, :],
                            xts[ic][v][
                                :, yb2 * 16 + ky : yb2 * 16 + ky + 16, :
                            ],
                            start=(ky == 0 and ic == 0),
                            stop=(ky == 2 and ic == 3),
                        )
                    if qi == 0 and oc == 0 and yb2 == 0:
                        # sigma's 16 tiny matmuls slot in here, after the
                        # first 48 conv matmuls (~18us in); their wsq/s2
                        # inputs arrive ~13us in, so little/no stall
                        emit_sigma()
                    emit_out(
                        b, h, oc, yb2, accs,
                        last=(
                            qi == len(quarters) - 1
                            and oc == NCH - 1
                            and yb2 == 1
                        ),
                    )
                if qi + 1 < len(quarters):
                    # spread the next quarter's x-prep across this
                    # quarter's oc iterations (one chunk per oc) so prep
                    # DVE work doesn't queue ahead of the PSUM-freeing
                    # output transforms
                    preps.setdefault(qi + 1, {}).update(
                        prep_half(*quarters[qi + 1], ics=(oc,))
                    )

    nc.compile()
    return nc


def get_nc(**kwargs):
    key = tuple(sorted(kwargs.items()))
    if key not in _NC_CACHE:
        _NC_CACHE[key] = _build(**kwargs)
    return _NC_CACHE[key]


# 1D Winograd F(2,3) filter transform (applied along kx)
_G = np.array(
    [[1, 0, 0], [0.5, 0.5, 0.5], [0.5, -0.5, 0.5], [0, 0, 1]], dtype=np.float64
)


def make_in_maps(x, s, weight):
    """Shard full inputs into 8 per-core input maps."""
    x = np.asarray(x, dtype=np.float32)
    s = np.asarray(s, dtype=np.float32)
    weight = np.asarray(weight, dtype=np.float32)
    # U[o,i,ky,v] = sum_kx G[v,kx] w[o,i,ky,kx]
    #   -> [128(i), oc, v*3+ky, ic, 128(o)], contiguous per oc
    u = np.einsum("vx,oiyx->oiyv", _G, weight.astype(np.float64))
    u_prep = np.ascontiguousarray(
        u.reshape(NCH, 128, NCH, 128, 3, 4)
        .transpose(3, 0, 5, 4, 2, 1)
        .reshape(128, NCH, 12, NCH, 128)
    ).astype(ml_dtypes.bfloat16)
    # wsq[i,o] = sum_kykx w^2  -> [128, NCH(ic), C(o)] f32
    wsq = (weight.astype(np.float64) ** 2).sum(axis=(2, 3)).T  # [i, o]
    wsq_prep = np.ascontiguousarray(
        wsq.reshape(NCH, 128, C).transpose(1, 0, 2)
    ).astype(np.float32)
    in_maps = []
    for core in range(N_CORES):
        xs = np.ascontiguousarray(x[core * B2 : (core + 1) * B2]).astype(
            ml_dtypes.bfloat16
        )
        ss = np.ascontiguousarray(
            s[core * B2 : (core + 1) * B2].reshape(B2, NCH, 128).transpose(2, 1, 0)
        )
        in_maps.append({"x": xs, "s": ss, "u": u_prep, "wsq": wsq_prep})
    return in_maps


def kernel(x, s, weight):
    nc = get_nc()
    in_maps = make_in_maps(x, s, weight)
    res = run_bass_kernel_spmd(nc, in_maps, list(range(N_CORES)))
    out = np.concatenate([r["o"] for r in res.results], axis=0)
    return out.astype(np.float32)


# revision 24
# speedup vs baseline: 1.1785x; 1.1785x over previous
"""ConvModLayer (StyleGAN2-style modulated 3x3 conv) on 8 Trainium2
NeuronCores — data-parallel over the batch (16 samples -> 2 per core).

Math (equivalent to the reference):
  cscale = 1/sqrt(512*9)
  s' = s * cscale
  sigma_sq[b,o] = sum_{i,ky,kx} (weight[o,i,ky,kx] * s'[b,i])^2
  out[b] = conv3x3(x[b] * s'[b,:,None,None], weight) * rsqrt(sigma_sq[b] + eps)

Device kernel (per core, identical SPMD program) — 1D Winograd F(2,3)
along x, direct conv along y, bf16 matmul operands:

  The kx-dimension (3 taps) is replaced by 4 Winograd products shared
  between each pair of output columns:
    V_v[c, y, bx] = BT-combos of padded x cols 2bx+0..3   (DVE, 4 ops/chunk)
    M_v[o, y, bx] = sum_{i,ky} U[o,i,ky,v] * V_v[i, y+ky, bx]  (PE, PSUM)
    out[:, 2bx+0] = (M0 + M1 + M2) * rsqrt(sigma)
    out[:, 2bx+1] = (M1 - M2 - M3) * rsqrt(sigma)
  MACs per output: 4*3*512 = 3072 vs direct 4608 -> 1.5x less PE time.

  U = w @ G^T (per ky) is sample-independent, computed on the HOST and
  shipped as a [128, 12(v*3+ky), 4, 512] bf16 input. The demod norm
  helper wsq[i,o] = sum_kykx w^2 is also host-computed (f32), so sigma
  on-device is just 16 tiny matmuls against s'^2 — no DVE reduction.

  PSUM: 4 M_v banks per 16-output-row group, two groups in flight.
"""

import sys
from contextlib import ExitStack

if "/opt/trn_rl_repo" not in sys.path:
    sys.path.insert(0, "/opt/trn_rl_repo")

import ml_dtypes
import numpy as np

import concourse.bacc as bacc
import concourse.mybir as mybir
import concourse.tile as tile
from concourse.bass_utils import run_bass_kernel_spmd

F32 = mybir.dt.float32
BF16 = mybir.dt.bfloat16

N_CORES = 8
B = 16
B2 = B // N_CORES  # samples per core
C = 512
NCH = 4  # 128-partition channel chunks
H = W = 64
EPS = 1e-8
CSCALE = 1.0 / (C * 9) ** 0.5

_NC_CACHE = {}


def _build(psum_bufs: int = 8, raw_bufs: int = 4):
    nc = bacc.Bacc("TRN2", target_bir_lowering=False, debug=False)

    x_d = nc.dram_tensor("x", [B2, C, H, W], BF16, kind="ExternalInput")
    s_d = nc.dram_tensor("s", [128, NCH, B2], F32, kind="ExternalInput")
    u_d = nc.dram_tensor("u", [128, NCH, 12, NCH, 128], BF16, kind="ExternalInput")
    wsq_d = nc.dram_tensor("wsq", [128, NCH, C], F32, kind="ExternalInput")
    o_d = nc.dram_tensor("o", [B2, C, H, W], F32, kind="ExternalOutput")

    with tile.TileContext(nc) as tc, ExitStack() as ctx:
        upool = ctx.enter_context(tc.tile_pool(name="upool", bufs=1))
        spool = ctx.enter_context(tc.tile_pool(name="spool", bufs=1))
        rawpool = ctx.enter_context(tc.tile_pool(name="rawpool", bufs=raw_bufs))
        ppool = ctx.enter_context(tc.tile_pool(name="ppool", bufs=1))
        vpool = ctx.enter_context(tc.tile_pool(name="vpool", bufs=2))
        tpool = ctx.enter_context(tc.tile_pool(name="tpool", bufs=2))
        opool = ctx.enter_context(tc.tile_pool(name="opool", bufs=4))
        pspool = ctx.enter_context(
            tc.tile_pool(name="pspool", bufs=psum_bufs, space="PSUM")
        )

        s_t = spool.tile([128, NCH, B2], F32)
        nc.sync.dma_start(s_t[:], s_d[:])
        nc.vector.tensor_scalar_mul(s_t[:], s_t[:], CSCALE)
        s2_t = spool.tile([128, NCH, B2], F32)
        nc.vector.tensor_mul(s2_t[:], s_t[:], s_t[:])

        # zeros for pad regions (f32 source for convert-copies)
        z66 = spool.tile([128, 66], F32)
        nc.vector.memset(z66[:], 0.0)

        def make_p(h, ic):
            """Padded modulated image tile [128, 34, 66] bf16: row zr and
            cols 0,65 are the conv zero-pad.

            NOTE: offloading these pads to GpSimd memsets and the scale
            to ACT was tried and REGRESSED 369->436us: the extra
            multi-engine SBUF traffic slowed the PE stream itself
            (matmul cadence 216->259ns). Keep prep on DVE."""
            zr = 0 if h == 0 else 33
            p = ppool.tile([128, 34, 66], BF16, tag=f"p{ic}", name="p")
            nc.vector.tensor_copy(p[:, zr, :], z66[:, 0:66])
            nc.vector.tensor_copy(p[:, :, 0], z66[:, 0:34])
            nc.vector.tensor_copy(p[:, :, 65], z66[:, 0:34])
            return p

        def scale_p(p, b, h, ic, raw, rows):
            r0, r1 = rows
            off = 1 if h == 0 else 0  # data rows start at this tile row
            nc.vector.tensor_scalar_mul(
                p[:, off + r0 : off + r1, 1:65],
                raw[:, r0:r1, :],
                s_t[:, ic, b : b + 1],
            )

        def alloc_v(ic):
            return [
                vpool.tile([128, 34, 32], BF16, tag=f"v{ic}_{v}", name="vt")
                for v in range(4)
            ]

        def fill_v(vts, p, rows):
            """Winograd column transform: V_v[y, bx] over padded cols
            2bx+v'. v0=d0-d2, v1=d1+d2, v2=d2-d1, v3=d1-d3."""
            r0, r1 = rows
            d0 = p[:, r0:r1, 0:64:2]
            d1 = p[:, r0:r1, 1:65:2]
            d2 = p[:, r0:r1, 2:66:2]
            d3 = p[:, r0:r1, 3:66:2]
            nc.vector.tensor_sub(vts[0][:, r0:r1, :], d0, d2)
            nc.vector.tensor_add(vts[1][:, r0:r1, :], d1, d2)
            nc.vector.tensor_sub(vts[2][:, r0:r1, :], d2, d1)
            nc.vector.tensor_sub(vts[3][:, r0:r1, :], d1, d3)

        def prep_half(b, h, ics=tuple(range(NCH))):
            xts = {}
            for ic in ics:
                raw = rawpool.tile([128, 33, 64], BF16, tag="raw", name="raw")
                r0 = 0 if h == 0 else 31
                nc.sync.dma_start(
                    raw[:], x_d[b, ic * 128 : (ic + 1) * 128, r0 : r0 + 33, :]
                )
                p = make_p(h, ic)
                scale_p(p, b, h, ic, raw, (0, 33))
                vts = alloc_v(ic)
                fill_v(vts, p, (0, 34))
                xts[ic] = vts
            return xts

        # Two hw DMA queues: x chunks + out stores on the SP (sync)
        # queue, U/wsq weights on the Activation queue — so the first
        # conv groups aren't gated on 6.3 MB of U behind x traffic.
        # U is laid out oc-major on the host so each oc's weights are
        # ONE contiguous 1.5 MB DMA (big packets); oc0 lands first and
        # covers the first two groups while the rest streams behind.
        u_oc = [
            upool.tile([128, 12, NCH, 128], BF16, tag=f"u{oc}", name="ut")
            for oc in range(NCH)
        ]
        # oc0 in three sub-DMAs so the first matmul gates on only the
        # first 0.5 MB rather than the full 1.5 MB transfer
        for vk0 in range(0, 12, 4):
            nc.scalar.dma_start(
                u_oc[0][:, vk0 : vk0 + 4], u_d[:, 0, vk0 : vk0 + 4]
            )
        wsq_t = spool.tile([128, NCH, C], F32)
        nc.scalar.dma_start(wsq_t[:], wsq_d[:])
        for oc in range(1, NCH):
            nc.scalar.dma_start(u_oc[oc][:], u_d[:, oc])

        # x chunk 0 rows 0..16 arrive first (the first matmul's gate);
        # rows 17..32 are only needed by the second group (~22us in), so
        # their DMA queues BEHIND chunks 1 and 2
        raw0 = rawpool.tile([128, 33, 64], BF16, tag="raw", name="raw")
        nc.sync.dma_start(raw0[:, 0:17], x_d[0, 0:128, 0:17, :])
        p0 = make_p(0, 0)
        scale_p(p0, 0, 0, 0, raw0, (0, 17))
        vts0 = alloc_v(0)
        fill_v(vts0, p0, (0, 18))
        xts_00 = {0: vts0}
        xts_00.update(prep_half(0, 0, ics=(1, 2)))
        nc.sync.dma_start(raw0[:, 17:33], x_d[0, 0:128, 17:33, :])
        xts_00.update(prep_half(0, 0, ics=(3,)))
        scale_p(p0, 0, 0, 0, raw0, (17, 33))
        fill_v(vts0, p0, (18, 34))

        # ---- sigma_sq[b, o] = sum_i wsq[i,o] * s'2[i,b] ----
        sig_t = spool.tile([128, NCH, B2], F32)

        def emit_sigma():
            # borrow a conv PSUM bank from the pool ring (same tag/shape)
            psig_t = pspool.tile([128, 16, 32], F32, tag="m", name="psig")
            for ic in range(NCH):
                for oc in range(NCH):
                    # start=True clears the WHOLE bank -> only the global
                    # first matmul sets it; later groups overwrite-where-
                    # unset via per-element has_written bits.
                    nc.tensor.matmul(
                        psig_t[:, oc, 0:B2],
                        wsq_t[:, ic, oc * 128 : (oc + 1) * 128],
                        s2_t[:, ic, :],
                        start=(ic == 0 and oc == 0),
                        stop=(ic == 3 and oc == 3),
                        skip_group_check=True,
                    )
            nc.vector.tensor_scalar_add(sig_t[:], psig_t[:, 0:NCH, 0:B2], EPS)
            nc.scalar.sqrt(sig_t[:], sig_t[:])
            nc.vector.reciprocal(sig_t[:], sig_t[:])

        # ---- conv: per sample, 2 halves of 32 rows = 2 groups of 16 ----
        quarters = [(b, h) for b in range(B2) for h in range(2)]
        preps = {0: xts_00}

        def emit_out(b, h, oc, yb2, accs, last=False, gi=[0]):
            # accs = [M0..M3] in PSUM, each [128, 16 rows, 32 bx]
            sig = sig_t[:, oc, b : b + 1]
            y0 = h * 32 + yb2 * 16
            # the very last group drains in two row-halves so the first
            # half's store overlaps the second half's transform
            halves = ((0, 8), (8, 16)) if last else ((0, 16),)
            for r0, r1 in halves:
                rr = r1 - r0
                c1 = tpool.tile([128, 16, 32], F32, tag="c1", name="c1")
                e1 = tpool.tile([128, 16, 32], F32, tag="e1", name="e1")
                e2 = tpool.tile([128, 16, 32], F32, tag="e2", name="e2")
                o1 = tpool.tile([128, 16, 32], F32, tag="o1", name="o1")
                o2 = tpool.tile([128, 16, 32], F32, tag="o2", name="o2")
                out_t = opool.tile([128, 16, 64], F32, tag="out", name="out")
                nc.vector.tensor_copy(c1[:, 0:rr, :], accs[1][:, r0:r1, :])
                nc.vector.tensor_add(e1[:, 0:rr, :], accs[0][:, r0:r1, :], c1[:, 0:rr, :])
                nc.vector.tensor_add(e2[:, 0:rr, :], e1[:, 0:rr, :], accs[2][:, r0:r1, :])
                nc.vector.tensor_sub(o1[:, 0:rr, :], c1[:, 0:rr, :], accs[2][:, r0:r1, :])
                nc.vector.tensor_sub(o2[:, 0:rr, :], o1[:, 0:rr, :], accs[3][:, r0:r1, :])
                nc.scalar.mul(out_t[:, 0:rr, 0::2], e2[:, 0:rr, :], sig)
                nc.scalar.mul(out_t[:, 0:rr, 1::2], o2[:, 0:rr, :], sig)
                # alternate store queues so out DMAs never back up behind
                # the next quarter's x loads (SP) or the U stream (ACT)
                eng = nc.sync if gi[0] % 2 == 0 else nc.scalar
                gi[0] += 1
                eng.dma_start(
                    o_d[b, oc * 128 : (oc + 1) * 128, y0 + r0 : y0 + r1, :],
                    out_t[:, 0:rr, :],
                )

        for qi, (b, h) in enumerate(quarters):
            xts = preps.pop(qi)
            for oc in range(NCH):
                for yb2 in range(2):
                    accs = [
                        pspool.tile([128, 16, 32], F32, tag="m", name=f"m{v}")
                        for v in range(4)
                    ]
                    # first two groups run ic-outer so early matmuls
                    # lean on x chunks that have already arrived; the
                    # steady state runs v-outer
                    if qi == 0 and oc == 0:
                        order = [
                            (v, ky, ic)
                            for ic in range(NCH)
                            for v in range(4)
                            for ky in range(3)
                        ]
                    else:
                        order = [
                            (v, ky, ic)
                            for v in range(4)
                            for ky in range(3)
                            for ic in range(NCH)
                        ]
                    for v, ky, ic in order:
                        nc.tensor.matmul(
                            accs[v][:],
                            u_oc[oc][:, v * 3 + ky, ic, :],
                            xts[ic][v][
                                :, yb2 * 16 + ky : yb2 * 16 + ky + 16, :
                            ],
                            start=(ky == 0 and ic == 0),
                            stop=(ky == 2 and ic == 3),
                        )
                    if qi == 0 and oc == 0 and yb2 == 0:
                        # sigma's 16 tiny matmuls slot in here, after the
                        # first 48 conv matmuls (~18us in); their wsq/s2
                        # inputs arrive ~13us in, so little/no stall
                        emit_sigma()
                    emit_out(
                        b, h, oc, yb2, accs,
                        last=(
                            qi == len(quarters) - 1
                            and oc == NCH - 1
                            and yb2 == 1
                        ),
                    )
                if qi + 1 < len(quarters):
                    # spread the next quarter's x-prep across this
                    # quarter's oc iterations (one chunk per oc) so prep
                    # DVE work doesn't queue ahead of the PSUM-freeing
                    # output transforms
                    preps.setdefault(qi + 1, {}).update(
                        prep_half(*quarters[qi + 1], ics=(oc,))
                    )

    nc.compile()
    return nc


def get_nc(**kwargs):
    key = tuple(sorted(kwargs.items()))
    if key not in _NC_CACHE:
        _NC_CACHE[key] = _build(**kwargs)
    return _NC_CACHE[key]


# 1D Winograd F(2,3) filter transform (applied along kx)
_G = np.array(
    [[1, 0, 0], [0.5, 0.5, 0.5], [0.5, -0.5, 0.5], [0, 0, 1]], dtype=np.float64
)


def make_in_maps(x, s, weight):
    """Shard full inputs into 8 per-core input maps."""
    x = np.asarray(x, dtype=np.float32)
    s = np.asarray(s, dtype=np.float32)
    weight = np.asarray(weight, dtype=np.float32)
    # U[o,i,ky,v] = sum_kx G[v,kx] w[o,i,ky,kx]
    #   -> [128(i), oc, v*3+ky, ic, 128(o)], contiguous per oc
    u = np.einsum("vx,oiyx->oiyv", _G, weight.astype(np.float64))
    u_prep = np.ascontiguousarray(
        u.reshape(NCH, 128, NCH, 128, 3, 4)
        .transpose(3, 0, 5, 4, 2, 1)
        .reshape(128, NCH, 12, NCH, 128)
    ).astype(ml_dtypes.bfloat16)
    # wsq[i,o] = sum_kykx w^2  -> [128, NCH(ic), C(o)] f32
    wsq = (weight.astype(np.float64) ** 2).sum(axis=(2, 3)).T  # [i, o]
    wsq_prep = np.ascontiguousarray(
        wsq.reshape(NCH, 128, C).transpose(1, 0, 2)
    ).astype(np.float32)
    in_maps = []
    for core in range(N_CORES):
        xs = np.ascontiguousarray(x[core * B2 : (core + 1) * B2]).astype(
            ml_dtypes.bfloat16
        )
        ss = np.ascontiguousarray(
            s[core * B2 : (core + 1) * B2].reshape(B2, NCH, 128).transpose(2, 1, 0)
        )
        in_maps.append({"x": xs, "s": ss, "u": u_prep, "wsq": wsq_prep})
    return in_maps


def kernel(x, s, weight):
    nc = get_nc()
    in_maps = make_in_maps(x, s, weight)
    res = run_bass_kernel_spmd(nc, in_maps, list(range(N_CORES)))
    out = np.concatenate([r["o"] for r in res.results], axis=0)
    return out.astype(np.float32)


# revision 26
# speedup vs baseline: 1.1913x; 1.0109x over previous
"""ConvModLayer (StyleGAN2-style modulated 3x3 conv) on 8 Trainium2
NeuronCores — data-parallel over the batch (16 samples -> 2 per core).

Math (equivalent to the reference):
  cscale = 1/sqrt(512*9)
  s' = s * cscale
  sigma_sq[b,o] = sum_{i,ky,kx} (weight[o,i,ky,kx] * s'[b,i])^2
  out[b] = conv3x3(x[b] * s'[b,:,None,None], weight) * rsqrt(sigma_sq[b] + eps)

Device kernel (per core, identical SPMD program) — 1D Winograd F(2,3)
along x, direct conv along y, bf16 matmul operands:

  The kx-dimension (3 taps) is replaced by 4 Winograd products shared
  between each pair of output columns:
    V_v[c, y, bx] = BT-combos of padded x cols 2bx+0..3   (DVE, 4 ops/chunk)
    M_v[o, y, bx] = sum_{i,ky} U[o,i,ky,v] * V_v[i, y+ky, bx]  (PE, PSUM)
    out[:, 2bx+0] = (M0 + M1 + M2) * rsqrt(sigma)
    out[:, 2bx+1] = (M1 - M2 - M3) * rsqrt(sigma)
  MACs per output: 4*3*512 = 3072 vs direct 4608 -> 1.5x less PE time.

  U = w @ G^T (per ky) is sample-independent, computed on the HOST and
  shipped as a [128, 12(v*3+ky), 4, 512] bf16 input. The demod norm
  helper wsq[i,o] = sum_kykx w^2 is also host-computed (f32), so sigma
  on-device is just 16 tiny matmuls against s'^2 — no DVE reduction.

  PSUM: 4 M_v banks per 16-output-row group, two groups in flight.
"""

import sys
from contextlib import ExitStack

if "/opt/trn_rl_repo" not in sys.path:
    sys.path.insert(0, "/opt/trn_rl_repo")

import ml_dtypes
import numpy as np

import concourse.bacc as bacc
import concourse.mybir as mybir
import concourse.tile as tile
from concourse.bass_utils import run_bass_kernel_spmd

F32 = mybir.dt.float32
BF16 = mybir.dt.bfloat16

N_CORES = 8
B = 16
B2 = B // N_CORES  # samples per core
C = 512
NCH = 4  # 128-partition channel chunks
H = W = 64
EPS = 1e-8
CSCALE = 1.0 / (C * 9) ** 0.5

_NC_CACHE = {}


def _build(psum_bufs: int = 8, raw_bufs: int = 3):
    nc = bacc.Bacc("TRN2", target_bir_lowering=False, debug=False)

    x_d = nc.dram_tensor("x", [B2, C, H, W], BF16, kind="ExternalInput")
    s_d = nc.dram_tensor("s", [128, NCH, B2], F32, kind="ExternalInput")
    u_d = nc.dram_tensor("u", [128, NCH, 12, NCH, 128], BF16, kind="ExternalInput")
    wsq_d = nc.dram_tensor("wsq", [128, NCH, C], F32, kind="ExternalInput")
    o_d = nc.dram_tensor("o", [B2, C, H, W], F32, kind="ExternalOutput")

    with tile.TileContext(nc) as tc, ExitStack() as ctx:
        upool = ctx.enter_context(tc.tile_pool(name="upool", bufs=1))
        spool = ctx.enter_context(tc.tile_pool(name="spool", bufs=1))
        rawpool = ctx.enter_context(tc.tile_pool(name="rawpool", bufs=raw_bufs))
        ppool = ctx.enter_context(tc.tile_pool(name="ppool", bufs=1))
        vpool = ctx.enter_context(tc.tile_pool(name="vpool", bufs=2))
        tpool = ctx.enter_context(tc.tile_pool(name="tpool", bufs=2))
        opool = ctx.enter_context(tc.tile_pool(name="opool", bufs=4))
        pspool = ctx.enter_context(
            tc.tile_pool(name="pspool", bufs=psum_bufs, space="PSUM")
        )

        s_t = spool.tile([128, NCH, B2], F32)
        nc.sync.dma_start(s_t[:], s_d[:])
        nc.vector.tensor_scalar_mul(s_t[:], s_t[:], CSCALE)
        s2_t = spool.tile([128, NCH, B2], F32)
        nc.vector.tensor_mul(s2_t[:], s_t[:], s_t[:])

        # zeros for pad regions (f32 source for convert-copies)
        z66 = spool.tile([128, 66], F32)
        nc.vector.memset(z66[:], 0.0)

        def make_p(h, ic):
            """Padded modulated image tile [128, 34, 66] bf16: row zr and
            cols 0,65 are the conv zero-pad.

            NOTE: offloading these pads to GpSimd memsets and the scale
            to ACT was tried and REGRESSED 369->436us: the extra
            multi-engine SBUF traffic slowed the PE stream itself
            (matmul cadence 216->259ns). Keep prep on DVE."""
            zr = 0 if h == 0 else 33
            p = ppool.tile([128, 34, 66], BF16, tag=f"p{ic}", name="p")
            nc.vector.tensor_copy(p[:, zr, :], z66[:, 0:66])
            nc.vector.tensor_copy(p[:, :, 0], z66[:, 0:34])
            nc.vector.tensor_copy(p[:, :, 65], z66[:, 0:34])
            return p

        def scale_p(p, b, h, ic, raw, rows):
            r0, r1 = rows
            off = 1 if h == 0 else 0  # data rows start at this tile row
            nc.vector.tensor_scalar_mul(
                p[:, off + r0 : off + r1, 1:65],
                raw[:, r0:r1, :],
                s_t[:, ic, b : b + 1],
            )

        def alloc_v(ic):
            return [
                vpool.tile([128, 34, 32], BF16, tag=f"v{ic}_{v}", name="vt")
                for v in range(4)
            ]

        def fill_v(vts, p, rows):
            """Winograd column transform: V_v[y, bx] over padded cols
            2bx+v'. v0=d0-d2, v1=d1+d2, v2=d2-d1, v3=d1-d3."""
            r0, r1 = rows
            d0 = p[:, r0:r1, 0:64:2]
            d1 = p[:, r0:r1, 1:65:2]
            d2 = p[:, r0:r1, 2:66:2]
            d3 = p[:, r0:r1, 3:66:2]
            nc.vector.tensor_sub(vts[0][:, r0:r1, :], d0, d2)
            nc.vector.tensor_add(vts[1][:, r0:r1, :], d1, d2)
            nc.vector.tensor_sub(vts[2][:, r0:r1, :], d2, d1)
            nc.vector.tensor_sub(vts[3][:, r0:r1, :], d1, d3)

        def prep_half(b, h, ics=tuple(range(NCH))):
            xts = {}
            for ic in ics:
                raw = rawpool.tile([128, 33, 64], BF16, tag="raw", name="raw")
                r0 = 0 if h == 0 else 31
                nc.sync.dma_start(
                    raw[:], x_d[b, ic * 128 : (ic + 1) * 128, r0 : r0 + 33, :]
                )
                p = make_p(h, ic)
                scale_p(p, b, h, ic, raw, (0, 33))
                vts = alloc_v(ic)
                fill_v(vts, p, (0, 34))
                xts[ic] = vts
            return xts

        # Two hw DMA queues: x chunks + out stores on the SP (sync)
        # queue, U/wsq weights on the Activation queue — so the first
        # conv groups aren't gated on 6.3 MB of U behind x traffic.
        # U is laid out oc-major on the host so each oc's weights are
        # ONE contiguous 1.5 MB DMA (big packets); oc0 lands first and
        # covers the first two groups while the rest streams behind.
        u_oc = [
            upool.tile([128, 12, NCH, 128], BF16, tag=f"u{oc}", name="ut")
            for oc in range(NCH)
        ]
        # oc0 in three sub-DMAs so the first matmul gates on only the
        # first 0.5 MB rather than the full 1.5 MB transfer
        for vk0 in range(0, 12, 4):
            nc.scalar.dma_start(
                u_oc[0][:, vk0 : vk0 + 4], u_d[:, 0, vk0 : vk0 + 4]
            )
        wsq_t = spool.tile([128, NCH, C], F32)
        nc.scalar.dma_start(wsq_t[:], wsq_d[:])
        for oc in range(1, NCH):
            nc.scalar.dma_start(u_oc[oc][:], u_d[:, oc])

        # first chunk split into two row-range DMAs so the DVE scale +
        # V-transform of rows 0..17 overlap the DMA of rows 17..32
        raw0 = rawpool.tile([128, 33, 64], BF16, tag="raw", name="raw")
        nc.sync.dma_start(raw0[:, 0:17], x_d[0, 0:128, 0:17, :])
        nc.sync.dma_start(raw0[:, 17:33], x_d[0, 0:128, 17:33, :])
        p0 = make_p(0, 0)
        scale_p(p0, 0, 0, 0, raw0, (0, 17))
        vts0 = alloc_v(0)
        fill_v(vts0, p0, (0, 18))
        scale_p(p0, 0, 0, 0, raw0, (17, 33))
        fill_v(vts0, p0, (18, 34))
        xts_00 = {0: vts0}
        xts_00.update(prep_half(0, 0, ics=(1, 2, 3)))

        # ---- sigma_sq[b, o] = sum_i wsq[i,o] * s'2[i,b] ----
        sig_t = spool.tile([128, NCH, B2], F32)

        def emit_sigma():
            # borrow a conv PSUM bank from the pool ring (same tag/shape)
            psig_t = pspool.tile([128, 16, 32], F32, tag="m", name="psig")
            for ic in range(NCH):
                for oc in range(NCH):
                    # start=True clears the WHOLE bank -> only the global
                    # first matmul sets it; later groups overwrite-where-
                    # unset via per-element has_written bits.
                    nc.tensor.matmul(
                        psig_t[:, oc, 0:B2],
                        wsq_t[:, ic, oc * 128 : (oc + 1) * 128],
                        s2_t[:, ic, :],
                        start=(ic == 0 and oc == 0),
                        stop=(ic == 3 and oc == 3),
                        skip_group_check=True,
                    )
            nc.vector.tensor_scalar_add(sig_t[:], psig_t[:, 0:NCH, 0:B2], EPS)
            nc.scalar.sqrt(sig_t[:], sig_t[:])
            nc.vector.reciprocal(sig_t[:], sig_t[:])

        # ---- conv: per sample, 2 halves of 32 rows = 2 groups of 16 ----
        quarters = [(b, h) for b in range(B2) for h in range(2)]
        preps = {0: xts_00}

        def emit_out(b, h, oc, yb2, accs, last=False, gi=[0]):
            # accs = [M0..M3] in PSUM, each [128, 16 rows, 32 bx]
            sig = sig_t[:, oc, b : b + 1]
            y0 = h * 32 + yb2 * 16
            # the very last group drains in two row-halves so the first
            # half's store overlaps the second half's transform
            halves = ((0, 8), (8, 16)) if last else ((0, 16),)
            for r0, r1 in halves:
                rr = r1 - r0
                c1 = tpool.tile([128, 16, 32], F32, tag="c1", name="c1")
                e1 = tpool.tile([128, 16, 32], F32, tag="e1", name="e1")
                e2 = tpool.tile([128, 16, 32], F32, tag="e2", name="e2")
                o1 = tpool.tile([128, 16, 32], F32, tag="o1", name="o1")
                o2 = tpool.tile([128, 16, 32], F32, tag="o2", name="o2")
                out_t = opool.tile([128, 16, 64], F32, tag="out", name="out")
                nc.vector.tensor_copy(c1[:, 0:rr, :], accs[1][:, r0:r1, :])
                nc.vector.tensor_add(e1[:, 0:rr, :], accs[0][:, r0:r1, :], c1[:, 0:rr, :])
                nc.vector.tensor_add(e2[:, 0:rr, :], e1[:, 0:rr, :], accs[2][:, r0:r1, :])
                nc.vector.tensor_sub(o1[:, 0:rr, :], c1[:, 0:rr, :], accs[2][:, r0:r1, :])
                nc.vector.tensor_sub(o2[:, 0:rr, :], o1[:, 0:rr, :], accs[3][:, r0:r1, :])
                nc.scalar.mul(out_t[:, 0:rr, 0::2], e2[:, 0:rr, :], sig)
                nc.scalar.mul(out_t[:, 0:rr, 1::2], o2[:, 0:rr, :], sig)
                # alternate store queues so out DMAs never back up behind
                # the next quarter's x loads (SP) or the U stream (ACT)
                eng = nc.sync if gi[0] % 2 == 0 else nc.scalar
                gi[0] += 1
                eng.dma_start(
                    o_d[b, oc * 128 : (oc + 1) * 128, y0 + r0 : y0 + r1, :],
                    out_t[:, 0:rr, :],
                )

        for qi, (b, h) in enumerate(quarters):
            xts = preps.pop(qi)
            for oc in range(NCH):
                for yb2 in range(2):
                    accs = [
                        pspool.tile([128, 16, 32], F32, tag="m", name=f"m{v}")
                        for v in range(4)
                    ]
                    # first two groups run ic-outer so early matmuls
                    # lean on x chunks that have already arrived; the
                    # steady state runs v-outer
                    if qi == 0 and oc == 0:
                        order = [
                            (v, ky, ic)
                            for ic in range(NCH)
                            for v in range(4)
                            for ky in range(3)
                        ]
                    else:
                        order = [
                            (v, ky, ic)
                            for v in range(4)
                            for ky in range(3)
                            for ic in range(NCH)
                        ]
                    for v, ky, ic in order:
                        nc.tensor.matmul(
                            accs[v][:],
                            u_oc[oc][:, v * 3 + ky, ic, :],
                            xts[ic][v][
                                :, yb2 * 16 + ky : yb2 * 16 + ky + 16, :
                            ],
                            start=(ky == 0 and ic == 0),
                            stop=(ky == 2 and ic == 3),
                        )
                    if qi == 0 and oc == 0 and yb2 == 0:
                        # sigma's 16 tiny matmuls slot in here, after the
                        # first 48 conv matmuls (~18us in); their wsq/s2
                        # inputs arrive ~13us in, so little/no stall
                        emit_sigma()
                    emit_out(
                        b, h, oc, yb2, accs,
                        last=(
                            qi == len(quarters) - 1
                            and oc == NCH - 1
                            and yb2 == 1
                        ),
                    )
                if qi + 1 < len(quarters):
                    # spread the next quarter's x-prep across this
                    # quarter's oc iterations (one chunk per oc) so prep
                    # DVE work doesn't queue ahead of the PSUM-freeing
                    # output transforms
                    preps.setdefault(qi + 1, {}).update(
                        prep_half(*quarters[qi + 1], ics=(oc,))
                    )

    nc.compile()
    return nc


def get_nc(**kwargs):
    key = tuple(sorted(kwargs.items()))
    if key not in _NC_CACHE:
        _NC_CACHE[key] = _build(**kwargs)
    return _NC_CACHE[key]


# 1D Winograd F(2,3) filter transform (applied along kx)
_G = np.array(
    [[1, 0, 0], [0.5, 0.5, 0.5], [0.5, -0.5, 0.5], [0, 0, 1]], dtype=np.float64
)


def make_in_maps(x, s, weight):
    """Shard full inputs into 8 per-core input maps."""
    x = np.asarray(x, dtype=np.float32)
    s = np.asarray(s, dtype=np.float32)
    weight = np.asarray(weight, dtype=np.float32)
    # U[o,i,ky,v] = sum_kx G[v,kx] w[o,i,ky,kx]
    #   -> [128(i), oc, v*3+ky, ic, 128(o)], contiguous per oc
    u = np.einsum("vx,oiyx->oiyv", _G, weight.astype(np.float64))
    u_prep = np.ascontiguousarray(
        u.reshape(NCH, 128, NCH, 128, 3, 4)
        .transpose(3, 0, 5, 4, 2, 1)
        .reshape(128, NCH, 12, NCH, 128)
    ).astype(ml_dtypes.bfloat16)
    # wsq[i,o] = sum_kykx w^2  -> [128, NCH(ic), C(o)] f32
    wsq = (weight.astype(np.float64) ** 2).sum(axis=(2, 3)).T  # [i, o]
    wsq_prep = np.ascontiguousarray(
        wsq.reshape(NCH, 128, C).transpose(1, 0, 2)
    ).astype(np.float32)
    in_maps = []
    for core in range(N_CORES):
        xs = np.ascontiguousarray(x[core * B2 : (core + 1) * B2]).astype(
            ml_dtypes.bfloat16
        )
        ss = np.ascontiguousarray(
            s[core * B2 : (core + 1) * B2].reshape(B2, NCH, 128).transpose(2, 1, 0)
        )
        in_maps.append({"x": xs, "s": ss, "u": u_prep, "wsq": wsq_prep})
    return in_maps


def kernel(x, s, weight):
    nc = get_nc()
    in_maps = make_in_maps(x, s, weight)
    res = run_bass_kernel_spmd(nc, in_maps, list(range(N_CORES)))
    out = np.concatenate([r["o"] for r in res.results], axis=0)
    return out.astype(np.float32)
